# revision 25
# baseline (speedup 1.0000x reference)
"""Trainium2 Bass kernel for bare-Coulomb GNN message passing.

potential[t] = 0.5 * sum_{pairs} 1/r * charges[s]  (both directions), computed as:
  - host: index-only layout — directed contributions (t <- s) are atom-sharded
    across 8 cores (12500 atoms/core), assigned to the SBUF partition owning t,
    sectioned by source bank (int16 indexing for the MoE dma_gather), t-sorted.
  - device per core: Ant dma_gather pulls charges rows (padded to 256 B) per
    contribution (2048-index calls, 2 SWDGE queues, 4096-desc rings); DVE
    computes 0.5/r * q and a per-partition chained prefix scan per channel;
    prefix is streamed to DRAM; per-atom run-end prefix values are read back
    with 16 bulk 256B-element dma_gathers (16 prefix rows per element) and a
    host-precomputed mod-16 one-hot mask selects the right row on DVE — this
    replaces 392 serial per-row indirect DMAs. Ends are differenced to yield
    per-atom sums; partition-prefix offsets cancel in the differences, so no
    cross-partition correction or collective is needed.
  - host: concatenate the 8 per-core outputs.

  Note: dma_gather index-list padding must use a valid row (0), NOT -1 —
  negative indices crash this path ("mesh desynced") despite the docstring.
"""
import numpy as np

N_ATOMS = 100000
N_CHANNELS = 4
NCORES = 8
AT_CORE = N_ATOMS // NCORES          # 12500
P = 128
R_AT = 98                            # atoms per partition row-grid (98*128=12544)
GRID = P * R_AT                      # 12544
BANKS = 4
BANK_ROWS = 25600                    # 4*25600 = 102400 >= N_ATOMS
CALL_NI = 4096                       # indices per dma_gather call
GCALLS = 2                           # gather calls per super-chunk
DMA_SCRATCH = 65536                  # SWDGE ring: 4096 descs/queue (1 call, exact fit
                                     # like the original 1024/16384 config)
SPC_G = CALL_NI // P                 # 8 slots per partition per gather call
SPC = SPC_G * GCALLS                 # 64 slots per partition per super-chunk
PAD_DIST = 1.0e30                    # pad slots get w ~ 1e-30 ~ 0

_CACHE = {}


def _preprocess(neighbor_indices, neighbor_distances):
    """Host-side index-metadata layout. Returns per-core input arrays + consts."""
    idx = np.asarray(neighbor_indices).astype(np.int64)
    dist = np.asarray(neighbor_distances).astype(np.float32)
    t = np.concatenate([idx[:, 0], idx[:, 1]])
    s = np.concatenate([idx[:, 1], idx[:, 0]])
    dd = np.concatenate([dist, dist])

    core = t // AT_CORE
    tl = t - core * AT_CORE                       # local atom id
    b = s // BANK_ROWS                            # source bank
    s16 = (s - b * BANK_ROWS).astype(np.int16)

    # Atom -> (partition, row) assignment: greedy per-bank load balancing.
    # Section sizes are padded to the max per-(core,p,bank) count, so
    # balancing the bank loads across partitions directly cuts pad
    # descriptors (the SWDGE descriptor-prep rate is the kernel bottleneck).
    ab = (core * AT_CORE + tl) * BANKS + b
    cnt_ab = np.bincount(ab, minlength=NCORES * AT_CORE * BANKS) \
               .reshape(NCORES, AT_CORE, BANKS)
    p_of = np.empty((NCORES, AT_CORE), dtype=np.int64)
    r_of = np.empty((NCORES, AT_CORE), dtype=np.int64)
    BIG = np.int64(1) << 40
    for k in range(NCORES):
        c4 = cnt_ab[k]
        order_a = np.argsort(-c4.max(axis=1), kind="stable")
        load = np.zeros((P, BANKS), dtype=np.int64)
        natoms = np.zeros(P, dtype=np.int64)
        for a in order_a:
            cand = (load + c4[a]).max(axis=1) + (natoms >= R_AT) * BIG
            pbest = int(np.argmin(cand))
            p_of[k, a] = pbest
            load[pbest] += c4[a]
            natoms[pbest] += 1
        ord2 = np.argsort(p_of[k], kind="stable")
        psorted = p_of[k][ord2]
        gstart = np.concatenate([[0], np.cumsum(np.bincount(psorted, minlength=P))[:-1]])
        r_of[k][ord2] = np.arange(AT_CORE) - gstart[psorted]
    g_of = p_of * R_AT + r_of                     # grid row of each local atom

    p = p_of[core, tl]                            # owning partition
    r = r_of[core, tl]

    # per (core, p, b) counts -> global section sizes
    gidx = (core * P + p) * BANKS + b
    cnt_pb = np.bincount(gidx, minlength=NCORES * P * BANKS).reshape(NCORES, P, BANKS)
    S_list = []
    for bb in range(BANKS):
        m = int(cnt_pb[:, :, bb].max())
        S_list.append(((m + SPC - 1) // SPC) * SPC)
    S = int(sum(S_list))
    base_b = np.concatenate([[0], np.cumsum(S_list)]).astype(np.int64)  # [5]
    ncalls = S // SPC
    bank_of_call = np.repeat(np.arange(BANKS), np.array(S_list) // SPC)

    # order contributions by (core, p, b, t); position within group
    order = np.lexsort((tl, b, p, core))
    co, po, bo, s16o, ddo, tlo, ro = (core[order], p[order], b[order],
                                      s16[order], dd[order], tl[order], r[order])
    g = (co * P + po) * BANKS + bo
    # start offset of each group in the sorted stream
    grp_counts = np.bincount(g, minlength=NCORES * P * BANKS)
    grp_starts = np.concatenate([[0], np.cumsum(grp_counts)[:-1]])
    within = np.arange(len(g)) - grp_starts[g]
    slot = base_b[bo] + within                    # slot within partition stream

    # per (core, p, b, r) cumulative end counts -> E positions
    g3 = ((co * P + po) * BANKS + bo) * R_AT + ro
    cnt3 = np.bincount(g3, minlength=NCORES * P * BANKS * R_AT) \
             .reshape(NCORES, P, BANKS, R_AT)
    endcnt = np.cumsum(cnt3, axis=3)              # inclusive
    # Eoff: per-bank prefix tensors, rows p*(S_b+1) + endcnt (+1 zero/carry col)
    pp = np.arange(P).reshape(1, P, 1, 1)
    sb1 = np.array([sl + 1 for sl in S_list]).reshape(1, 1, BANKS, 1)
    eoff = pp * sb1 + endcnt
    eoff = eoff.astype(np.int32)                  # [NCORES, P, BANKS, R_AT]

    # Boundary bulk-gather metadata: the per-(b, r) prefix row eoff is read
    # back with 256B-element dma_gathers (16 rows of the [*, 4] prefix tensor
    # per j per call; wrap i = j_local*128 + p lands j on all partitions) and
    # a mod-16 one-hot mask selects the right 16B row within each element.
    JCALL = CALL_NI // P                          # j's per gather call (16)
    BCALL_B = (R_AT + JCALL - 1) // JCALL         # calls per bank (7)
    BCALLS = BANKS * BCALL_B                      # 28
    NJP_B = BCALL_B * JCALL                       # 112 padded j's per bank
    bidx_all, msk_all = [], []
    for k in range(NCORES):
        rows = eoff[k]                            # [P, BANKS, R_AT]
        bidx_k = np.zeros((BCALLS, CALL_NI), dtype=np.int64)
        msk_k = np.zeros((P, BCALLS * JCALL, 16), dtype=np.float32)
        for b in range(BANKS):
            for c in range(BCALL_B):
                jlo = c * JCALL
                jhi = min(jlo + JCALL, R_AT)
                call = b * BCALL_B + c
                for jl in range(jlo, jhi):
                    i0 = (jl - jlo) * P
                    rr = rows[:, b, jl]           # [P] row into [P*(S_b+1), 4]
                    bidx_k[call, i0:i0 + P] = rr // 16
                    msk_k[np.arange(P), call * JCALL + (jl - jlo), rr % 16] = 1.0
        assert bidx_k.max() < 32768
        # wrap each call's 2048 indices into the 16-partition dma_gather format
        w16 = bidx_k.astype(np.int16).reshape(BCALLS, CALL_NI // 16, 16) \
                    .transpose(0, 2, 1)
        wr_b = np.tile(w16, (1, 8, 1)).reshape(BCALLS * P, CALL_NI // 16)
        bidx_all.append(np.ascontiguousarray(wr_b))
        msk_all.append(msk_k)

    per_core = []
    for k in range(NCORES):
        m = co == k
        pk, sk, dk, slk = po[m], s16o[m], ddo[m], slot[m]
        src = np.zeros((P, S), dtype=np.int16)
        dts = np.full((P, S), PAD_DIST, dtype=np.float32)
        src[pk, slk] = sk
        dts[pk, slk] = dk
        # wrapped idx tiles: 1024-idx gather calls, 8 per super-chunk.
        ng = ncalls * GCALLS  # number of 1024-idx gather calls
        src_calls = src.reshape(P, ng, SPC_G).transpose(1, 0, 2)   # [ng, P, 8]
        lists = src_calls.transpose(0, 2, 1).reshape(ng, CALL_NI)  # k2 = p + 128*u
        w16 = lists.reshape(ng, CALL_NI // 16, 16).transpose(0, 2, 1)  # [ng,16,64]
        wr = np.tile(w16, (1, 8, 1))                                # [ng, 128, 64]
        wrapped = wr.reshape(ncalls, GCALLS, P, CALL_NI // 16) \
                    .transpose(0, 2, 1, 3).reshape(ncalls * P, GCALLS * (CALL_NI // 16))
        per_core.append({
            "idx": wrapped,
            "dist": dts,
            "bidx": bidx_all[k],
            "bmsk": msk_all[k],
            "gout": g_of[k],
        })
    consts = {"S": S, "S_list": S_list, "ncalls": ncalls,
              "bank_of_call": bank_of_call.tolist()}
    return per_core, consts


def _pad_table(charges):
    tab = np.zeros((BANKS * BANK_ROWS, 64), dtype=np.float32)
    tab[:N_ATOMS, :N_CHANNELS] = np.asarray(charges, dtype=np.float32)
    return tab


def _build_bass(S, ncalls, bank_of_call, S_list, repeat=1):
    import os
    import concourse.bacc as bacc
    import concourse.tile as tile
    import concourse.bass as bass
    from concourse import mybir
    bisect = os.environ.get("KBISECT", "full")

    NJ = BANKS * R_AT  # 392 boundary values per partition

    nc = bacc.Bacc("TRN2", target_bir_lowering=False, debug=False,
                   num_devices=NCORES, num_swdge_queues=2,
                   dynamic_dma_scratch_size=DMA_SCRATCH)
    table = nc.dram_tensor("table", [BANKS * BANK_ROWS, 64], mybir.dt.float32,
                           kind="ExternalInput").ap()
    idx_d = nc.dram_tensor("idx", [ncalls * P, GCALLS * (CALL_NI // 16)], mybir.dt.int16,
                           kind="ExternalInput").ap()
    dist_d = nc.dram_tensor("dist", [P, S], mybir.dt.float32,
                            kind="ExternalInput").ap()
    JCALL = CALL_NI // P
    BCALL_B = (R_AT + JCALL - 1) // JCALL
    BCALLS = BANKS * BCALL_B
    bidx_d = nc.dram_tensor("bidx", [BCALLS * P, CALL_NI // 16], mybir.dt.int16,
                            kind="ExternalInput").ap()
    bmsk_d = nc.dram_tensor("bmsk", [P, BCALLS * JCALL, 16], mybir.dt.float32,
                            kind="ExternalInput").ap()
    prefixes = [nc.dram_tensor(f"prefix{b}", [P * (S_list[b] + 1), N_CHANNELS],
                               mybir.dt.float32, kind="Internal").ap()
                for b in range(BANKS)]
    out_d = nc.dram_tensor("out", [GRID, N_CHANNELS], mybir.dt.float32,
                           kind="ExternalOutput").ap()

    prefix3s = [pr.rearrange("(p s) c -> p s c", p=P) for pr in prefixes]
    call_base = []
    acc = 0
    for b in range(BANKS):
        call_base.append(acc)
        acc += S_list[b] // SPC
    out3 = out_d.rearrange("(p r) c -> p r c", p=P)

    with tile.TileContext(nc) as tc:
        with tc.tile_pool(name="idxp", bufs=6) as idxp, \
             tc.tile_pool(name="gp", bufs=4) as gp, \
             tc.tile_pool(name="wp", bufs=4) as wp, \
             tc.tile_pool(name="pfp", bufs=4) as pfp, \
             tc.tile_pool(name="bgp", bufs=2) as bgp, \
             tc.tile_pool(name="persist", bufs=1) as pers:

            zt = pers.tile([P, 1, N_CHANNELS], mybir.dt.float32)
            nc.gpsimd.memset(zt[:], 0.0)
            nc.sync.dma_start(prefix3s[0][:, 0:1, :], zt[:])

            carry = pers.tile([P, N_CHANNELS], mybir.dt.float32)

            bidxs = pers.tile([P, BCALLS, CALL_NI // 16], mybir.dt.int16)
            nc.sync.dma_start(
                bidxs[:], bidx_d.rearrange("(b p) w -> p b w", p=P))

            et = pers.tile([P, NJ * N_CHANNELS], mybir.dt.float32)
            dt2 = pers.tile([P, NJ * N_CHANNELS], mybir.dt.float32)
            SEC = R_AT * N_CHANNELS
            o1 = pers.tile([P, SEC], mybir.dt.float32)
            o2 = pers.tile([P, SEC], mybir.dt.float32)
            ot = pers.tile([P, R_AT, N_CHANNELS], mybir.dt.float32)

            for rep in range(repeat):
              nc.gpsimd.memset(carry[:], 0.0)
              for c in range(ncalls):
                bk = bank_of_call[c]
                it = idxp.tile([P, GCALLS * (CALL_NI // 16)], mybir.dt.int16, tag="it")
                nc.sync.dma_start(it[:], idx_d[c * P:(c + 1) * P, :])
                g = gp.tile([P, SPC, 64], mybir.dt.float32, tag="g")
                if "nogather" in bisect:
                    nc.gpsimd.memset(g[:], 1.0)
                else:
                    W16 = CALL_NI // 16
                    for ci in range(GCALLS):
                        nc.gpsimd.dma_gather(
                            out_ap=g[:, ci * SPC_G:(ci + 1) * SPC_G, :],
                            in_ap=table[bk * BANK_ROWS:(bk + 1) * BANK_ROWS, :],
                            idxs_ap=it[:, ci * W16:(ci + 1) * W16],
                            num_idxs=CALL_NI, num_idxs_reg=CALL_NI, elem_size=64,
                            single_packet=False, queue_num=ci % 2,
                        )
                dt_ = wp.tile([P, SPC], mybir.dt.float32, tag="dt")
                nc.sync.dma_start(dt_[:], dist_d[:, c * SPC:(c + 1) * SPC])
                wt = wp.tile([P, SPC], mybir.dt.float32, tag="wt")
                nc.vector.reciprocal(wt[:], dt_[:])
                v = wp.tile([P, SPC, N_CHANNELS], mybir.dt.float32, tag="v")
                nc.vector.scalar_tensor_tensor(
                    out=v[:], in0=g[:, :, 0:N_CHANNELS], scalar=0.5,
                    in1=wt[:, :, None].to_broadcast([P, SPC, N_CHANNELS]),
                    op0=mybir.AluOpType.mult, op1=mybir.AluOpType.mult,
                )
                pf = pfp.tile([P, SPC, N_CHANNELS], mybir.dt.float32, tag="pf")
                for ch in range(N_CHANNELS):
                    nc.vector.tensor_tensor_scan(
                        out=pf[:, :, ch], data0=v[:, :, ch], data1=v[:, :, ch],
                        initial=carry[:, ch:ch + 1],
                        op0=mybir.AluOpType.add, op1=mybir.AluOpType.bypass,
                    )
                nc.vector.tensor_copy(carry[:], pf[:, SPC - 1, :])
                cl = c - call_base[bk]
                nc.sync.dma_start(
                    prefix3s[bk][:, 1 + cl * SPC:1 + (cl + 1) * SPC, :], pf[:])
                if bk + 1 < BANKS and c + 1 == call_base[bk + 1]:
                    nc.sync.dma_start(prefix3s[bk + 1][:, 0:1, :],
                                      pf[:, SPC - 1:SPC, :])

              if "noj" in bisect:
                  nc.gpsimd.memset(et[:], 0.0)
              else:
                  W16 = CALL_NI // 16
                  etv = et[:].rearrange("p (j c) -> p j c", c=N_CHANNELS)
                  for b in range(BANKS):
                      pview = prefixes[b].rearrange("(r x) c -> r (x c)", x=16)
                      for cc in range(BCALL_B):
                          call = b * BCALL_B + cc
                          jlo, jhi = cc * JCALL, min((cc + 1) * JCALL, R_AT)
                          jn = jhi - jlo
                          G = bgp.tile([P, JCALL, 64], mybir.dt.float32, tag="G")
                          nc.gpsimd.dma_gather(
                              out_ap=G[:], in_ap=pview,
                              idxs_ap=bidxs[:, call, :],
                              num_idxs=CALL_NI, num_idxs_reg=CALL_NI,
                              elem_size=64, single_packet=False,
                              queue_num=call % 2)
                          mt = bgp.tile([P, JCALL, 16], mybir.dt.float32, tag="mt")
                          nc.sync.dma_start(
                              mt[:], bmsk_d[:, call * JCALL:(call + 1) * JCALL, :])
                          Gv = G[:].rearrange("p j (m c) -> p j m c", c=N_CHANNELS)
                          for ch in range(N_CHANNELS):
                              tm = bgp.tile([P, JCALL, 16], mybir.dt.float32, tag="tm")
                              nc.vector.tensor_tensor(
                                  out=tm[:, :jn, :],
                                  in0=Gv[:, :jn, :, ch], in1=mt[:, :jn, :],
                                  op=mybir.AluOpType.mult)
                              nc.vector.tensor_reduce(
                                  out=etv[:, b * R_AT + jlo:b * R_AT + jhi, ch],
                                  in_=tm[:, :jn, :],
                                  axis=mybir.AxisListType.X,
                                  op=mybir.AluOpType.add)
              nc.vector.tensor_copy(dt2[:, 0:4], et[:, 0:4])
              nc.vector.tensor_tensor(
                  out=dt2[:, 4:], in0=et[:, 4:], in1=et[:, 0:NJ * 4 - 4],
                  op=mybir.AluOpType.subtract)
              nc.vector.tensor_add(o1[:], dt2[:, 0:SEC], dt2[:, SEC:2 * SEC])
              nc.vector.tensor_add(o2[:], dt2[:, 2 * SEC:3 * SEC], dt2[:, 3 * SEC:4 * SEC])
              o1v = o1[:].rearrange("p (r c) -> p r c", c=N_CHANNELS)
              o2v = o2[:].rearrange("p (r c) -> p r c", c=N_CHANNELS)
              nc.vector.tensor_add(ot[:], o1v, o2v)
              nc.sync.dma_start(out3[:, :, :], ot[:])
    nc.finalize()
    return nc


class _Runner:
    def __init__(self, nc, n_cores):
        import jax
        from jax.sharding import Mesh, PartitionSpec
        try:
            from jax.experimental.shard_map import shard_map
        except Exception:
            from jax.sharding import shard_map
        from concourse import mybir
        from concourse.bass2jax import (_bass_exec_p, partition_id_tensor,
                                        install_neuronx_cc_hook)
        install_neuronx_cc_hook()
        self.jax = jax
        self.n_cores = n_cores
        pname = nc.partition_id_tensor.name if nc.partition_id_tensor else None
        in_names, out_names, out_avals, zero_outs = [], [], [], []
        for alloc in nc.m.functions[0].allocations:
            if not isinstance(alloc, mybir.MemoryLocationSet):
                continue
            name = alloc.memorylocations[0].name
            if alloc.kind == "ExternalInput":
                if name != pname:
                    in_names.append(name)
            elif alloc.kind == "ExternalOutput":
                shape = tuple(alloc.tensor_shape)
                dtype = mybir.dt.np(alloc.dtype)
                out_names.append(name)
                out_avals.append(jax.core.ShapedArray(shape, dtype))
                zero_outs.append(np.zeros(shape, dtype))
        self.in_names, self.out_names = in_names, out_names
        self.out_avals, self.zero_outs = out_avals, zero_outs
        n_params, n_outs = len(in_names), len(out_names)
        all_in = list(in_names) + list(out_names)
        if pname is not None:
            all_in.append(pname)

        def _body(*args):
            operands = list(args)
            if pname is not None:
                operands.append(partition_id_tensor())
            outs = _bass_exec_p.bind(
                *operands, out_avals=tuple(out_avals), in_names=tuple(all_in),
                out_names=tuple(out_names), lowering_input_output_aliases=(),
                sim_require_finite=False, sim_require_nnan=False, nc=nc)
            return tuple(outs)

        devices = jax.devices()[:n_cores]
        mesh = Mesh(np.asarray(devices), ("core",))
        in_specs = (PartitionSpec("core"),) * (n_params + n_outs)
        out_specs = (PartitionSpec("core"),) * n_outs
        from jax.sharding import NamedSharding
        self.sharding = NamedSharding(mesh, PartitionSpec("core"))
        self.fn = jax.jit(
            shard_map(_body, mesh=mesh, in_specs=in_specs,
                      out_specs=out_specs, check_rep=False),
            keep_unused=True)

    def device_args(self, in_maps):
        """Shard-place concatenated inputs on the cores (outside timed region)."""
        jax = self.jax
        concat_in = [
            np.concatenate([np.asarray(in_maps[c][n]) for c in range(self.n_cores)], axis=0)
            for n in self.in_names]
        concat_zeros = [
            np.zeros((self.n_cores * z.shape[0], *z.shape[1:]), z.dtype)
            for z in self.zero_outs]
        dargs = [jax.device_put(a, self.sharding) for a in concat_in + concat_zeros]
        jax.block_until_ready(dargs)
        return dargs

    def run(self, in_maps):
        jax = self.jax
        dargs = self.device_args(in_maps)
        outs = self.fn(*dargs)
        jax.block_until_ready(outs)
        res = []
        for c in range(self.n_cores):
            d = {}
            for i, n in enumerate(self.out_names):
                a = np.asarray(outs[i]).reshape(self.n_cores, *self.out_avals[i].shape)
                d[n] = a[c]
            res.append(d)
        return res


def _build_args(consts):
    return (consts["S"], consts["ncalls"], consts["bank_of_call"],
            consts["S_list"])


def _make_in_maps(charges, per_core):
    tab = _pad_table(charges)
    return [{"table": tab, "idx": pc["idx"], "dist": pc["dist"],
             "bidx": pc["bidx"], "bmsk": pc["bmsk"]} for pc in per_core]


def kernel(charges, cell, positions, neighbor_indices, neighbor_distances):
    per_core, consts = _preprocess(neighbor_indices, neighbor_distances)
    key = (consts["S"], consts["ncalls"], tuple(consts["bank_of_call"]))
    if key not in _CACHE:
        nc = _build_bass(*_build_args(consts))
        _CACHE[key] = _Runner(nc, NCORES)
    runner = _CACHE[key]
    in_maps = _make_in_maps(charges, per_core)
    res = runner.run(in_maps)
    out = np.concatenate(
        [res[k]["out"][per_core[k]["gout"]] for k in range(NCORES)], axis=0)
    return out.astype(np.float32)



# revision 26
# speedup vs baseline: 1.0594x; 1.0594x over previous
"""Trainium2 Bass kernel for bare-Coulomb GNN message passing.

potential[t] = 0.5 * sum_{pairs} 1/r * charges[s]  (both directions), computed as:
  - host: index-only layout — directed contributions (t <- s) are atom-sharded
    across 8 cores (12500 atoms/core), assigned to the SBUF partition owning t,
    sectioned by source bank (int16 indexing for the MoE dma_gather), t-sorted.
  - device per core: Ant dma_gather pulls charges rows (padded to 256 B) per
    contribution (2048-index calls, 2 SWDGE queues, 4096-desc rings); DVE
    computes 0.5/r * q and a per-partition chained prefix scan per channel;
    prefix is streamed to DRAM; per-atom run-end prefix values are read back
    with 16 bulk 256B-element dma_gathers (16 prefix rows per element) and a
    host-precomputed mod-16 one-hot mask selects the right row on DVE — this
    replaces 392 serial per-row indirect DMAs. Ends are differenced to yield
    per-atom sums; partition-prefix offsets cancel in the differences, so no
    cross-partition correction or collective is needed.
  - host: concatenate the 8 per-core outputs.

  Note: dma_gather index-list padding must use a valid row (0), NOT -1 —
  negative indices crash this path ("mesh desynced") despite the docstring.
"""
import numpy as np

N_ATOMS = 100000
N_CHANNELS = 4
NCORES = 8
AT_CORE = N_ATOMS // NCORES          # 12500
P = 128
R_AT = 98                            # atoms per partition row-grid (98*128=12544)
GRID = P * R_AT                      # 12544
BANKS = 4
BANK_ROWS = 25600                    # 4*25600 = 102400 >= N_ATOMS
CALL_NI = 4096                       # indices per dma_gather call
GCALLS = 2                           # gather calls per super-chunk
DMA_SCRATCH = 65536                  # SWDGE ring: 4096 descs/queue (1 call, exact fit
                                     # like the original 1024/16384 config)
SPC_G = CALL_NI // P                 # 8 slots per partition per gather call
SPC = SPC_G * GCALLS                 # 64 slots per partition per super-chunk
PAD_DIST = 1.0e30                    # pad slots get w ~ 1e-30 ~ 0

_CACHE = {}


def _preprocess(neighbor_indices, neighbor_distances):
    """Host-side index-metadata layout. Returns per-core input arrays + consts."""
    idx = np.asarray(neighbor_indices).astype(np.int64)
    dist = np.asarray(neighbor_distances).astype(np.float32)
    t = np.concatenate([idx[:, 0], idx[:, 1]])
    s = np.concatenate([idx[:, 1], idx[:, 0]])
    dd = np.concatenate([dist, dist])

    core = t // AT_CORE
    tl = t - core * AT_CORE                       # local atom id
    b = s // BANK_ROWS                            # source bank
    s16 = (s - b * BANK_ROWS).astype(np.int16)

    # Atom -> (partition, row) assignment: greedy per-bank load balancing.
    # Section sizes are padded to the max per-(core,p,bank) count, so
    # balancing the bank loads across partitions directly cuts pad
    # descriptors (the SWDGE descriptor-prep rate is the kernel bottleneck).
    ab = (core * AT_CORE + tl) * BANKS + b
    cnt_ab = np.bincount(ab, minlength=NCORES * AT_CORE * BANKS) \
               .reshape(NCORES, AT_CORE, BANKS)
    p_of = np.empty((NCORES, AT_CORE), dtype=np.int64)
    r_of = np.empty((NCORES, AT_CORE), dtype=np.int64)
    BIG = np.int64(1) << 40
    for k in range(NCORES):
        c4 = cnt_ab[k]
        order_a = np.argsort(-c4.max(axis=1), kind="stable")
        load = np.zeros((P, BANKS), dtype=np.int64)
        natoms = np.zeros(P, dtype=np.int64)
        for a in order_a:
            cand = (load + c4[a]).max(axis=1) + (natoms >= R_AT) * BIG
            pbest = int(np.argmin(cand))
            p_of[k, a] = pbest
            load[pbest] += c4[a]
            natoms[pbest] += 1
        ord2 = np.argsort(p_of[k], kind="stable")
        psorted = p_of[k][ord2]
        gstart = np.concatenate([[0], np.cumsum(np.bincount(psorted, minlength=P))[:-1]])
        r_of[k][ord2] = np.arange(AT_CORE) - gstart[psorted]
    g_of = p_of * R_AT + r_of                     # grid row of each local atom

    p = p_of[core, tl]                            # owning partition
    r = r_of[core, tl]

    # per (core, p, b) counts -> global section sizes
    gidx = (core * P + p) * BANKS + b
    cnt_pb = np.bincount(gidx, minlength=NCORES * P * BANKS).reshape(NCORES, P, BANKS)
    S_list = []
    for bb in range(BANKS):
        m = int(cnt_pb[:, :, bb].max())
        S_list.append(((m + SPC - 1) // SPC) * SPC)
    S = int(sum(S_list))
    base_b = np.concatenate([[0], np.cumsum(S_list)]).astype(np.int64)  # [5]
    ncalls = S // SPC
    bank_of_call = np.repeat(np.arange(BANKS), np.array(S_list) // SPC)

    # order contributions by (core, p, b, t); position within group
    order = np.lexsort((tl, b, p, core))
    co, po, bo, s16o, ddo, tlo, ro = (core[order], p[order], b[order],
                                      s16[order], dd[order], tl[order], r[order])
    g = (co * P + po) * BANKS + bo
    # start offset of each group in the sorted stream
    grp_counts = np.bincount(g, minlength=NCORES * P * BANKS)
    grp_starts = np.concatenate([[0], np.cumsum(grp_counts)[:-1]])
    within = np.arange(len(g)) - grp_starts[g]
    slot = base_b[bo] + within                    # slot within partition stream

    # per (core, p, b, r) cumulative end counts -> E positions
    g3 = ((co * P + po) * BANKS + bo) * R_AT + ro
    cnt3 = np.bincount(g3, minlength=NCORES * P * BANKS * R_AT) \
             .reshape(NCORES, P, BANKS, R_AT)
    endcnt = np.cumsum(cnt3, axis=3)              # inclusive
    # Eoff: per-bank prefix tensors, rows p*(S_b+1) + endcnt (+1 zero/carry col)
    pp = np.arange(P).reshape(1, P, 1, 1)
    sb1 = np.array([sl + 1 for sl in S_list]).reshape(1, 1, BANKS, 1)
    eoff = pp * sb1 + endcnt
    eoff = eoff.astype(np.int32)                  # [NCORES, P, BANKS, R_AT]

    # Boundary bulk-gather metadata: the per-(b, r) prefix row eoff is read
    # back with 256B-element dma_gathers (16 rows of the [*, 4] prefix tensor
    # per j per call; wrap i = j_local*128 + p lands j on all partitions) and
    # a mod-16 one-hot mask selects the right 16B row within each element.
    JCALL = CALL_NI // P                          # j's per gather call (16)
    BCALL_B = (R_AT + JCALL - 1) // JCALL         # calls per bank (7)
    BCALLS = BANKS * BCALL_B                      # 28
    NJP_B = BCALL_B * JCALL                       # 112 padded j's per bank
    bidx_all, msk_all = [], []
    for k in range(NCORES):
        rows = eoff[k]                            # [P, BANKS, R_AT]
        bidx_k = np.zeros((BCALLS, CALL_NI), dtype=np.int64)
        msk_k = np.zeros((P, BCALLS * JCALL, 16), dtype=np.float32)
        for b in range(BANKS):
            for c in range(BCALL_B):
                jlo = c * JCALL
                jhi = min(jlo + JCALL, R_AT)
                call = b * BCALL_B + c
                for jl in range(jlo, jhi):
                    i0 = (jl - jlo) * P
                    rr = rows[:, b, jl]           # [P] row into [P*(S_b+1), 4]
                    bidx_k[call, i0:i0 + P] = rr // 16
                    msk_k[np.arange(P), call * JCALL + (jl - jlo), rr % 16] = 1.0
        assert bidx_k.max() < 32768
        # wrap each call's 2048 indices into the 16-partition dma_gather format
        w16 = bidx_k.astype(np.int16).reshape(BCALLS, CALL_NI // 16, 16) \
                    .transpose(0, 2, 1)
        wr_b = np.tile(w16, (1, 8, 1)).reshape(BCALLS * P, CALL_NI // 16)
        bidx_all.append(np.ascontiguousarray(wr_b))
        msk_all.append(msk_k)

    per_core = []
    for k in range(NCORES):
        m = co == k
        pk, sk, dk, slk = po[m], s16o[m], ddo[m], slot[m]
        src = np.zeros((P, S), dtype=np.int16)
        dts = np.full((P, S), PAD_DIST, dtype=np.float32)
        src[pk, slk] = sk
        dts[pk, slk] = dk
        # wrapped idx tiles: 1024-idx gather calls, 8 per super-chunk.
        ng = ncalls * GCALLS  # number of 1024-idx gather calls
        src_calls = src.reshape(P, ng, SPC_G).transpose(1, 0, 2)   # [ng, P, 8]
        lists = src_calls.transpose(0, 2, 1).reshape(ng, CALL_NI)  # k2 = p + 128*u
        w16 = lists.reshape(ng, CALL_NI // 16, 16).transpose(0, 2, 1)  # [ng,16,64]
        wr = np.tile(w16, (1, 8, 1))                                # [ng, 128, 64]
        wrapped = wr.reshape(ncalls, GCALLS, P, CALL_NI // 16) \
                    .transpose(0, 2, 1, 3).reshape(ncalls * P, GCALLS * (CALL_NI // 16))
        per_core.append({
            "idx": wrapped,
            "dist": dts,
            "bidx": bidx_all[k],
            "bmsk": msk_all[k],
            "gout": g_of[k],
        })
    consts = {"S": S, "S_list": S_list, "ncalls": ncalls,
              "bank_of_call": bank_of_call.tolist()}
    return per_core, consts


def _pad_table(charges):
    tab = np.zeros((BANKS * BANK_ROWS, 64), dtype=np.float32)
    tab[:N_ATOMS, :N_CHANNELS] = np.asarray(charges, dtype=np.float32)
    return tab


def _build_bass(S, ncalls, bank_of_call, S_list, repeat=1):
    import os
    import concourse.bacc as bacc
    import concourse.tile as tile
    import concourse.bass as bass
    from concourse import mybir
    bisect = os.environ.get("KBISECT", "full")

    NJ = BANKS * R_AT  # 392 boundary values per partition

    nc = bacc.Bacc("TRN2", target_bir_lowering=False, debug=False,
                   num_devices=NCORES, num_swdge_queues=2,
                   dynamic_dma_scratch_size=DMA_SCRATCH)
    table = nc.dram_tensor("table", [BANKS * BANK_ROWS, 64], mybir.dt.float32,
                           kind="ExternalInput").ap()
    idx_d = nc.dram_tensor("idx", [ncalls * P, GCALLS * (CALL_NI // 16)], mybir.dt.int16,
                           kind="ExternalInput").ap()
    dist_d = nc.dram_tensor("dist", [P, S], mybir.dt.float32,
                            kind="ExternalInput").ap()
    JCALL = CALL_NI // P
    BCALL_B = (R_AT + JCALL - 1) // JCALL
    BCALLS = BANKS * BCALL_B
    bidx_d = nc.dram_tensor("bidx", [BCALLS * P, CALL_NI // 16], mybir.dt.int16,
                            kind="ExternalInput").ap()
    bmsk_d = nc.dram_tensor("bmsk", [P, BCALLS * JCALL, 16], mybir.dt.float32,
                            kind="ExternalInput").ap()
    prefixes = [nc.dram_tensor(f"prefix{b}", [P * (S_list[b] + 1), N_CHANNELS],
                               mybir.dt.float32, kind="Internal").ap()
                for b in range(BANKS)]
    out_d = nc.dram_tensor("out", [GRID, N_CHANNELS], mybir.dt.float32,
                           kind="ExternalOutput").ap()

    prefix3s = [pr.rearrange("(p s) c -> p s c", p=P) for pr in prefixes]
    call_base = []
    acc = 0
    for b in range(BANKS):
        call_base.append(acc)
        acc += S_list[b] // SPC
    out3 = out_d.rearrange("(p r) c -> p r c", p=P)

    with tile.TileContext(nc) as tc:
        with tc.tile_pool(name="idxp", bufs=6) as idxp, \
             tc.tile_pool(name="gp", bufs=4) as gp, \
             tc.tile_pool(name="wp", bufs=4) as wp, \
             tc.tile_pool(name="pfp", bufs=4) as pfp, \
             tc.tile_pool(name="bgp", bufs=2) as bgp, \
             tc.tile_pool(name="persist", bufs=1) as pers:

            zt = pers.tile([P, 1, N_CHANNELS], mybir.dt.float32)
            nc.gpsimd.memset(zt[:], 0.0)
            nc.sync.dma_start(prefix3s[0][:, 0:1, :], zt[:])

            carry = pers.tile([P, N_CHANNELS], mybir.dt.float32)

            bidxs = pers.tile([P, BCALLS, CALL_NI // 16], mybir.dt.int16)
            nc.sync.dma_start(
                bidxs[:], bidx_d.rearrange("(b p) w -> p b w", p=P))

            et = pers.tile([P, NJ * N_CHANNELS], mybir.dt.float32)
            dt2 = pers.tile([P, NJ * N_CHANNELS], mybir.dt.float32)
            SEC = R_AT * N_CHANNELS
            o1 = pers.tile([P, SEC], mybir.dt.float32)
            o2 = pers.tile([P, SEC], mybir.dt.float32)
            ot = pers.tile([P, R_AT, N_CHANNELS], mybir.dt.float32)

            for rep in range(repeat):
              nc.gpsimd.memset(carry[:], 0.0)
              for c in range(ncalls):
                bk = bank_of_call[c]
                it = idxp.tile([P, GCALLS * (CALL_NI // 16)], mybir.dt.int16, tag="it")
                nc.sync.dma_start(it[:], idx_d[c * P:(c + 1) * P, :])
                g = gp.tile([P, SPC, 64], mybir.dt.float32, tag="g")
                if "nogather" in bisect:
                    nc.gpsimd.memset(g[:], 1.0)
                else:
                    W16 = CALL_NI // 16
                    for ci in range(GCALLS):
                        nc.gpsimd.dma_gather(
                            out_ap=g[:, ci * SPC_G:(ci + 1) * SPC_G, :],
                            in_ap=table[bk * BANK_ROWS:(bk + 1) * BANK_ROWS, :],
                            idxs_ap=it[:, ci * W16:(ci + 1) * W16],
                            num_idxs=CALL_NI, num_idxs_reg=CALL_NI, elem_size=64,
                            single_packet=False, queue_num=ci % 2,
                        )
                dt_ = wp.tile([P, SPC], mybir.dt.float32, tag="dt")
                nc.sync.dma_start(dt_[:], dist_d[:, c * SPC:(c + 1) * SPC])
                wt = wp.tile([P, SPC], mybir.dt.float32, tag="wt")
                nc.vector.reciprocal(wt[:], dt_[:])
                v = wp.tile([P, SPC, N_CHANNELS], mybir.dt.float32, tag="v")
                nc.vector.scalar_tensor_tensor(
                    out=v[:], in0=g[:, :, 0:N_CHANNELS], scalar=0.5,
                    in1=wt[:, :, None].to_broadcast([P, SPC, N_CHANNELS]),
                    op0=mybir.AluOpType.mult, op1=mybir.AluOpType.mult,
                )
                pf = pfp.tile([P, SPC, N_CHANNELS], mybir.dt.float32, tag="pf")
                for ch in range(N_CHANNELS):
                    nc.vector.tensor_tensor_scan(
                        out=pf[:, :, ch], data0=v[:, :, ch], data1=v[:, :, ch],
                        initial=carry[:, ch:ch + 1],
                        op0=mybir.AluOpType.add, op1=mybir.AluOpType.bypass,
                    )
                nc.vector.tensor_copy(carry[:], pf[:, SPC - 1, :])
                cl = c - call_base[bk]
                nc.sync.dma_start(
                    prefix3s[bk][:, 1 + cl * SPC:1 + (cl + 1) * SPC, :], pf[:])
                if bk + 1 < BANKS and c + 1 == call_base[bk + 1]:
                    nc.sync.dma_start(prefix3s[bk + 1][:, 0:1, :],
                                      pf[:, SPC - 1:SPC, :])

              if "noj" in bisect:
                  nc.gpsimd.memset(et[:], 0.0)
              else:
                  W16 = CALL_NI // 16
                  etv = et[:].rearrange("p (j c) -> p j c", c=N_CHANNELS)
                  for b in range(BANKS):
                      pview = prefixes[b].rearrange("(r x) c -> r (x c)", x=16)
                      for cc in range(BCALL_B):
                          call = b * BCALL_B + cc
                          jlo, jhi = cc * JCALL, min((cc + 1) * JCALL, R_AT)
                          jn = jhi - jlo
                          # last call per bank: only jn*P real indices; the wrap
                          # is position-preserving so they occupy the first
                          # jn*P//16 idx columns — issue a short call.
                          ni = jn * P
                          G = bgp.tile([P, ni // P, 64], mybir.dt.float32, tag="G")
                          nc.gpsimd.dma_gather(
                              out_ap=G[:], in_ap=pview,
                              idxs_ap=bidxs[:, call, :ni // 16],
                              num_idxs=ni, num_idxs_reg=ni,
                              elem_size=64, single_packet=False,
                              queue_num=call % 2)
                          mt = bgp.tile([P, JCALL, 16], mybir.dt.float32, tag="mt")
                          nc.sync.dma_start(
                              mt[:], bmsk_d[:, call * JCALL:(call + 1) * JCALL, :])
                          Gv = G[:].rearrange("p j (m c) -> p j m c", c=N_CHANNELS)
                          for ch in range(N_CHANNELS):
                              tm = bgp.tile([P, JCALL, 16], mybir.dt.float32, tag="tm")
                              nc.vector.tensor_tensor(
                                  out=tm[:, :jn, :],
                                  in0=Gv[:, :jn, :, ch], in1=mt[:, :jn, :],
                                  op=mybir.AluOpType.mult)
                              nc.vector.tensor_reduce(
                                  out=etv[:, b * R_AT + jlo:b * R_AT + jhi, ch],
                                  in_=tm[:, :jn, :],
                                  axis=mybir.AxisListType.X,
                                  op=mybir.AluOpType.add)
              nc.vector.tensor_copy(dt2[:, 0:4], et[:, 0:4])
              nc.vector.tensor_tensor(
                  out=dt2[:, 4:], in0=et[:, 4:], in1=et[:, 0:NJ * 4 - 4],
                  op=mybir.AluOpType.subtract)
              nc.vector.tensor_add(o1[:], dt2[:, 0:SEC], dt2[:, SEC:2 * SEC])
              nc.vector.tensor_add(o2[:], dt2[:, 2 * SEC:3 * SEC], dt2[:, 3 * SEC:4 * SEC])
              o1v = o1[:].rearrange("p (r c) -> p r c", c=N_CHANNELS)
              o2v = o2[:].rearrange("p (r c) -> p r c", c=N_CHANNELS)
              nc.vector.tensor_add(ot[:], o1v, o2v)
              nc.sync.dma_start(out3[:, :, :], ot[:])
    nc.finalize()
    return nc


class _Runner:
    def __init__(self, nc, n_cores):
        import jax
        from jax.sharding import Mesh, PartitionSpec
        try:
            from jax.experimental.shard_map import shard_map
        except Exception:
            from jax.sharding import shard_map
        from concourse import mybir
        from concourse.bass2jax import (_bass_exec_p, partition_id_tensor,
                                        install_neuronx_cc_hook)
        install_neuronx_cc_hook()
        self.jax = jax
        self.n_cores = n_cores
        pname = nc.partition_id_tensor.name if nc.partition_id_tensor else None
        in_names, out_names, out_avals, zero_outs = [], [], [], []
        for alloc in nc.m.functions[0].allocations:
            if not isinstance(alloc, mybir.MemoryLocationSet):
                continue
            name = alloc.memorylocations[0].name
            if alloc.kind == "ExternalInput":
                if name != pname:
                    in_names.append(name)
            elif alloc.kind == "ExternalOutput":
                shape = tuple(alloc.tensor_shape)
                dtype = mybir.dt.np(alloc.dtype)
                out_names.append(name)
                out_avals.append(jax.core.ShapedArray(shape, dtype))
                zero_outs.append(np.zeros(shape, dtype))
        self.in_names, self.out_names = in_names, out_names
        self.out_avals, self.zero_outs = out_avals, zero_outs
        n_params, n_outs = len(in_names), len(out_names)
        all_in = list(in_names) + list(out_names)
        if pname is not None:
            all_in.append(pname)

        def _body(*args):
            operands = list(args)
            if pname is not None:
                operands.append(partition_id_tensor())
            outs = _bass_exec_p.bind(
                *operands, out_avals=tuple(out_avals), in_names=tuple(all_in),
                out_names=tuple(out_names), lowering_input_output_aliases=(),
                sim_require_finite=False, sim_require_nnan=False, nc=nc)
            return tuple(outs)

        devices = jax.devices()[:n_cores]
        mesh = Mesh(np.asarray(devices), ("core",))
        in_specs = (PartitionSpec("core"),) * (n_params + n_outs)
        out_specs = (PartitionSpec("core"),) * n_outs
        from jax.sharding import NamedSharding
        self.sharding = NamedSharding(mesh, PartitionSpec("core"))
        self.fn = jax.jit(
            shard_map(_body, mesh=mesh, in_specs=in_specs,
                      out_specs=out_specs, check_rep=False),
            keep_unused=True)

    def device_args(self, in_maps):
        """Shard-place concatenated inputs on the cores (outside timed region)."""
        jax = self.jax
        concat_in = [
            np.concatenate([np.asarray(in_maps[c][n]) for c in range(self.n_cores)], axis=0)
            for n in self.in_names]
        concat_zeros = [
            np.zeros((self.n_cores * z.shape[0], *z.shape[1:]), z.dtype)
            for z in self.zero_outs]
        dargs = [jax.device_put(a, self.sharding) for a in concat_in + concat_zeros]
        jax.block_until_ready(dargs)
        return dargs

    def run(self, in_maps):
        jax = self.jax
        dargs = self.device_args(in_maps)
        outs = self.fn(*dargs)
        jax.block_until_ready(outs)
        res = []
        for c in range(self.n_cores):
            d = {}
            for i, n in enumerate(self.out_names):
                a = np.asarray(outs[i]).reshape(self.n_cores, *self.out_avals[i].shape)
                d[n] = a[c]
            res.append(d)
        return res


def _build_args(consts):
    return (consts["S"], consts["ncalls"], consts["bank_of_call"],
            consts["S_list"])


def _make_in_maps(charges, per_core):
    tab = _pad_table(charges)
    return [{"table": tab, "idx": pc["idx"], "dist": pc["dist"],
             "bidx": pc["bidx"], "bmsk": pc["bmsk"]} for pc in per_core]


def kernel(charges, cell, positions, neighbor_indices, neighbor_distances):
    per_core, consts = _preprocess(neighbor_indices, neighbor_distances)
    key = (consts["S"], consts["ncalls"], tuple(consts["bank_of_call"]))
    if key not in _CACHE:
        nc = _build_bass(*_build_args(consts))
        _CACHE[key] = _Runner(nc, NCORES)
    runner = _CACHE[key]
    in_maps = _make_in_maps(charges, per_core)
    res = runner.run(in_maps)
    out = np.concatenate(
        [res[k]["out"][per_core[k]["gout"]] for k in range(NCORES)], axis=0)
    return out.astype(np.float32)

